# revision 20
# baseline (speedup 1.0000x reference)
"""Trainium2 Bass kernel for nn_EntityAttentionLayer (sparse attention).

Strategy (8 cores, data-parallel over bs):
  - Host side: shard bs across 8 cores (64 items each), pre-transpose
    entities to E^T[in_dim, ne] per batch, cast operands to bf16,
    convert masks to multiplicative keep-masks.
  - Query compaction: rows killed by post_mask are never computed.
    Host gathers the unmasked query entities (and their pre_mask rows)
    per batch, padded to PAD (multiple of 4, >= max unmasked count).
    All q-sized work (Q proj, logits, softmax, attn, out proj) shrinks
    64 -> PAD; the post-mask multiply disappears (kept rows are exactly
    the unmasked ones). Host scatters outputs back, zeros elsewhere.
  - On chip, per batch b (processed in pairs, Q in octets of 8):
      K^T[ed, ne]  = (Wk^T E^T)        via lhsT=Wk slices, rhs=E^T
      V[ne, ed]    = E V-proj          via lhsT=E^T slices, rhs=Wv
      Q^T[ed, q]   =                   via lhsT=Wq slices, rhs=Eq^T
      logits^T[ne, q] per head-pair    lhsT=K^T_h, rhs=Q^T (2 heads, N=2*PAD)
      wm = exp(logits * 1/sqrt(hd))    on ACT (scale folded into activation)
      wm *= keep^T                     on DVE (multiplicative mask, h-broadcast)
      sums broadcast [128, h*PAD]      PE matmul with all-ones lhsT
      attn^T unnorm [2-heads, hp*b*q]  lhsT=V slices, rhs=wm  (col-tiled pairs)
      attn = attn_unnorm * 1/sums      DVE (approx reciprocal + strided muls)
      out[b*q, out] = attn^T.T @ W_out (+bias on DVE), DMA out.
  All matmuls bf16 operands, fp32 PSUM accumulation.
"""

import numpy as np
import ml_dtypes

BS, NE, NQ, IN_DIM, ED, OUT_DIM, H, HD = 512, 256, 64, 512, 512, 512, 8, 64
NCORES = 8
BPC = BS // NCORES          # 64 batches per core
OCT = 8                     # batches per super-batch (Q^T amortization)
NOCT = BPC // OCT           # 8
PAIRS_PER_OCT = OCT // 2    # 4
NPAIRS = BPC // 2           # 32
SCALE = 1.0 / float(np.sqrt(HD))

BF16 = ml_dtypes.bfloat16

_BUILT = {}
LAST_RESULT = None


def _build_nc(PAD):
    import concourse.tile as tile
    from concourse import bacc, mybir
    from contextlib import ExitStack

    f32 = mybir.dt.float32
    bf16 = mybir.dt.bfloat16
    HQ = H * PAD            # logits/wm free size per (b2, n2)
    P2 = 2 * PAD            # head-pair logits block

    nc = bacc.Bacc("TRN2", target_bir_lowering=False)

    eT_d = nc.dram_tensor("eT", [NOCT, 128, OCT, 4, NE], bf16, kind="ExternalInput")
    eq_d = nc.dram_tensor("eq", [NOCT, 128, OCT, 4, PAD], bf16, kind="ExternalInput")
    keep_d = nc.dram_tensor("keep", [NOCT, 128, OCT, 2, PAD], bf16, kind="ExternalInput")
    w_in_d = nc.dram_tensor("w_in", [4, 128, 3 * ED], bf16, kind="ExternalInput")
    w_out_d = nc.dram_tensor("w_out", [4, 128, OUT_DIM], bf16, kind="ExternalInput")
    b_out_d = nc.dram_tensor("b_out", [1, OUT_DIM], f32, kind="ExternalInput")
    out_d = nc.dram_tensor("out", [BPC, PAD, OUT_DIM], bf16, kind="ExternalOutput")

    with ExitStack() as ctx:
        tc = ctx.enter_context(tile.TileContext(nc))
        consts = ctx.enter_context(tc.tile_pool(name="consts", bufs=1))
        p_eT = ctx.enter_context(tc.tile_pool(name="p_eT", bufs=3))
        p_eq = ctx.enter_context(tc.tile_pool(name="p_eq", bufs=3))
        p_keep = ctx.enter_context(tc.tile_pool(name="p_keep", bufs=3))
        p_kT = ctx.enter_context(tc.tile_pool(name="p_kT", bufs=3))
        p_v = ctx.enter_context(tc.tile_pool(name="p_v", bufs=3))
        p_wm = ctx.enter_context(tc.tile_pool(name="p_wm", bufs=6))
        p_recip = ctx.enter_context(tc.tile_pool(name="p_recip", bufs=2))
        p_attn = ctx.enter_context(tc.tile_pool(name="p_attn", bufs=2))
        p_out = ctx.enter_context(tc.tile_pool(name="p_out", bufs=3))
        pp = ctx.enter_context(tc.tile_pool(name="pp", bufs=2, space="PSUM"))

        # Constants. Q weights first (the first matmul needs them), in
        # 256-col chunks alternating issue engines to spread DMA queues.
        w_sb = consts.tile([128, 4, 3 * ED], bf16)
        engs = (nc.scalar, nc.gpsimd)
        for i in range(12):
            engs[i % 2].dma_start(
                out=w_sb[:, :, i * 128:(i + 1) * 128],
                in_=w_in_d[:, :, i * 128:(i + 1) * 128].rearrange("k p n -> p k n"))
        wo_sb = consts.tile([128, 4, OUT_DIM], bf16)
        nc.gpsimd.dma_start(out=wo_sb, in_=w_out_d[:, :, :].rearrange("k p n -> p k n"))
        bias_bc = consts.tile([128, OUT_DIM], f32)
        nc.gpsimd.dma_start(out=bias_bc, in_=b_out_d[:, :].to_broadcast([128, OUT_DIM]))
        ones_sb = consts.tile([128, 128], bf16)
        nc.vector.memset(ones_sb, 1.0)
        # Persistent zero-padded Q^T tiles (manual double buffer by octet
        # parity). Layout [128, m, h2, b, q]: head parity h2 selects which
        # 64-row half holds data; the other half stays zero so logits
        # matmuls can use full K=128 operands at base partition 0
        # (operands at base partition 64 fault on HW).
        qz0 = consts.tile([128, 4, 2, OCT, PAD], bf16)
        nc.vector.memset(qz0, 0.0)
        qz1 = consts.tile([128, 4, 2, OCT, PAD], bf16)
        nc.vector.memset(qz1, 0.0)
        qz_bufs = [qz0, qz1]

        def flush_out(pend):
            # out-projection of a PREVIOUS pair, emitted one pair late
            # so the PE never waits on that pair's DVE normalize.
            attn_prev, gp = pend
            ps_o = pp.tile([128, OUT_DIM], f32, tag="attn", name="ps_o")
            for t in range(4):
                nc.tensor.matmul(
                    ps_o[0:P2, :],
                    lhsT=attn_prev[:, t * P2:(t + 1) * P2],
                    rhs=wo_sb[:, t, :],
                    start=(t == 0),
                    stop=(t == 3),
                )
            out_sb = p_out.tile([128, OUT_DIM], bf16, tag="out_sb")
            nc.vector.tensor_add(out_sb[0:P2, :], ps_o[0:P2, :], bias_bc[0:P2, :])
            for i in range(2):
                nc.sync.dma_start(
                    out=out_d[gp * 2 + i],
                    in_=out_sb[i * PAD:(i + 1) * PAD, :],
                )

        pending = None
        for oc in range(NOCT):
            # finer-grained DMAs on the cold first octet: per-queue DMA
            # bandwidth is ~20 GB/s, so startup latency is set by the
            # largest single transfer on the critical path.
            eqc = 4 if oc == 0 else 1
            etc = 8 if oc == 0 else 4
            eq_sb = p_eq.tile([128, OCT, 4, PAD], bf16, tag="eq")
            for i in range(eqc):
                nc.sync.dma_start(
                    out=eq_sb[:, i * (OCT // eqc):(i + 1) * (OCT // eqc), :, :],
                    in_=eq_d[oc, :, i * (OCT // eqc):(i + 1) * (OCT // eqc), :, :])
            eT_sb = p_eT.tile([128, OCT, 4, NE], bf16, tag="eT")
            for hc in range(etc):
                w = OCT // etc
                nc.sync.dma_start(
                    out=eT_sb[:, hc * w:(hc + 1) * w, :, :],
                    in_=eT_d[oc, :, hc * w:(hc + 1) * w, :, :],
                )
            keep_sb = p_keep.tile([128, OCT, 2, PAD], bf16, tag="keep")
            nc.sync.dma_start(out=keep_sb, in_=keep_d[oc, :, :, :, :])

            # ---- Q^T for the whole octet: amortize W_q weight loads ----
            qz = qz_bufs[oc % 2]
            for m in range(4):
                # logit-tag ring: keeps the K-proj's proj-ring allocation
                # independent of the qz copy drain at octet boundaries.
                ps_q = pp.tile([128, OCT * PAD], f32, tag="logit", name="ps_q", bufs=2)
                for k in range(4):
                    nc.tensor.matmul(
                        ps_q,
                        lhsT=w_sb[:, k, m * 128:(m + 1) * 128],
                        rhs=eq_sb[:, :, k, :],
                        start=(k == 0),
                        stop=(k == 3),
                    )
                nc.scalar.copy(out=qz[0:64, m, 0, :, :], in_=ps_q[0:64, :])
                nc.vector.tensor_copy(out=qz[64:128, m, 1, :, :], in_=ps_q[64:128, :])

            for pr in range(PAIRS_PER_OCT):
                lb = pr * 2          # local batch index in octet
                gpair = oc * PAIRS_PER_OCT + pr

                # ---- K^T ----
                kT_sb = p_kT.tile([128, 4, 2, NE], bf16, tag="kT")
                for m in range(4):
                    ps_k = pp.tile([128, 2 * NE], f32, tag="proj", name="ps_k", bufs=3)
                    for k in range(4):
                        nc.tensor.matmul(
                            ps_k,
                            lhsT=w_sb[:, k, ED + m * 128:ED + (m + 1) * 128],
                            rhs=eT_sb[:, lb:lb + 2, k, :],
                            start=(k == 0),
                            stop=(k == 3),
                        )
                    if m % 2 == 0:
                        nc.vector.tensor_copy(out=kT_sb[:, m, :, :], in_=ps_k)
                    else:
                        nc.scalar.copy(out=kT_sb[:, m, :, :], in_=ps_k)

                # ---- V ----
                v_sb = p_v.tile([128, 2, 2, ED], bf16, tag="v")
                for n2 in range(2):
                    for b2 in range(2):
                        ps_v = pp.tile([128, ED], f32, tag="proj", name="ps_v", bufs=3)
                        for k in range(4):
                            nc.tensor.matmul(
                                ps_v,
                                lhsT=eT_sb[:, lb + b2, k, n2 * 128:(n2 + 1) * 128],
                                rhs=w_sb[:, k, 2 * ED:3 * ED],
                                start=(k == 0),
                                stop=(k == 3),
                            )
                        if n2 == 0:
                            nc.vector.tensor_copy(out=v_sb[:, n2, b2, :], in_=ps_v)
                        else:
                            nc.scalar.copy(out=v_sb[:, n2, b2, :], in_=ps_v)

                # previous pair's output projection goes here (PE slack)
                if pending is not None:
                    flush_out(pending)
                    pending = None

                # ---- logits^T + exp + keep-mask ----
                # wm[(b2, n2)] : [128(ne-slice), H*PAD] bf16
                # b2-outer so each batch's GPSIMD wmsum pre-add can start
                # right after its second keep-mul; the PE's attn matmuls
                # below then hide the gpsimd latency before sums.
                wm = {}
                wmsum = {}
                for b2 in range(2):
                    for n2 in range(2):
                        ps_l = pp.tile([128, HQ], f32, tag="logit", name="ps_l", bufs=2)
                        for m in range(4):
                            nc.tensor.matmul(
                                ps_l[:, m * P2:(m + 1) * P2],
                                lhsT=kT_sb[:, m, b2, n2 * 128:(n2 + 1) * 128],
                                rhs=qz[:, m, :, lb + b2, :],
                                start=True,
                                stop=True,
                            )
                        wm_t = p_wm.tile([128, HQ], bf16, tag="wm", name="wm_t")
                        nc.scalar.activation(
                            out=wm_t, in_=ps_l,
                            func=mybir.ActivationFunctionType.Exp,
                            scale=SCALE,
                        )
                        keep_rep = keep_sb[:, lb + b2, n2, None, :].broadcast_to(
                            [128, H, PAD])
                        nc.vector.tensor_mul(wm_t, wm_t, keep_rep)
                        wm[(b2, n2)] = wm_t
                    ws = p_wm.tile([128, HQ], bf16, tag="wmsum", name="wmsum")
                    nc.gpsimd.tensor_add(ws, wm[(b2, 0)], wm[(b2, 1)])
                    wmsum[b2] = ws

                # ---- attn (unnormalized) ----
                ps_a = pp.tile([128, 4 * P2], f32, tag="attn", name="ps_a")
                for hp in range(4):
                    for b2 in range(2):
                        for h2 in range(2):
                            h = 2 * hp + h2
                            col = (hp * 2 + b2) * PAD
                            for n2 in range(2):
                                nc.tensor.matmul(
                                    ps_a[h2 * 64:(h2 + 1) * 64, col:col + PAD],
                                    lhsT=v_sb[:, n2, b2, h * 64:(h + 1) * 64],
                                    rhs=wm[(b2, n2)][:, h * PAD:(h + 1) * PAD],
                                    start=(n2 == 0),
                                    stop=(n2 == 1),
                                )

                # ---- softmax denominators (after attn: gpsimd slack) ----
                recip = {}
                for b2 in range(2):
                    ps_s = pp.tile([128, HQ], f32, tag="sums", name="ps_s", bufs=1)
                    nc.tensor.matmul(
                        ps_s, lhsT=ones_sb, rhs=wmsum[b2], start=True, stop=True)
                    r_sb = p_recip.tile([128, HQ], f32, tag="recip", name="r_sb")
                    nc.vector.reciprocal_approx_fast(out=r_sb, in_=ps_s)
                    recip[b2] = r_sb

                # normalize -> attn_sb (bf16), layout [128(2-head rows), (hp, b2, q)]
                attn_sb = p_attn.tile([128, 4 * P2], bf16, tag="attn_sb")
                for b2 in range(2):
                    for h2 in range(2):
                        rows = slice(h2 * 64, (h2 + 1) * 64)
                        o_ap = attn_sb[rows, :].rearrange(
                            "p (hp b q) -> p hp b q", hp=4, b=2)[:, :, b2, :]
                        i_ap = ps_a[rows, :].rearrange(
                            "p (hp b q) -> p hp b q", hp=4, b=2)[:, :, b2, :]
                        r_ap = recip[b2][rows, :].rearrange(
                            "p (hp x) -> p hp x", hp=4)[:, :, h2 * PAD:(h2 + 1) * PAD]
                        nc.vector.tensor_mul(o_ap, i_ap, r_ap)

                pending = (attn_sb, gpair)
        flush_out(pending)
    nc.compile()
    return nc


def _prep_inputs(entities, pre_mask, post_mask, W_in, W_out, b_out, PAD):
    """Host-side sharding + layout transforms (not part of timed HW work)."""
    # Query compaction: per batch, gather the post_mask==0 query rows,
    # padded (edge-repeat) to PAD. Padded columns recompute the last
    # kept query, so device output for them equals the kept row's
    # output; the scatter in kernel() only reads the first cnt rows.
    keepq = (post_mask == 0)
    cnt = keepq.sum(axis=1)
    idx = np.zeros((BS, PAD), dtype=np.int64)
    for b in range(BS):
        ib = np.nonzero(keepq[b])[0]
        if ib.size == 0:
            ib = np.zeros(1, dtype=np.int64)
        idx[b, :ib.size] = ib
        idx[b, ib.size:] = ib[-1]
    ar = np.arange(BS)[:, None]
    eqg = entities[ar, idx, :]                          # [BS, PAD, 512]
    kg = (1 - pre_mask).astype(np.float32)[ar, idx, :]  # [BS, PAD, 256]

    # [oct, 128p, b, k, ne] contiguous per partition row
    eT = entities.reshape(BS // OCT, OCT, NE, 4, 128).transpose(
        0, 4, 1, 3, 2).astype(BF16)
    eT = np.ascontiguousarray(eT)
    eq = eqg.reshape(BS // OCT, OCT, PAD, 4, 128).transpose(
        0, 4, 1, 3, 2).astype(BF16)
    eq = np.ascontiguousarray(eq)
    keep = kg.reshape(BS // OCT, OCT, PAD, 2, 128).transpose(
        0, 4, 1, 3, 2).astype(BF16)
    keep = np.ascontiguousarray(keep)
    w_in = np.ascontiguousarray(W_in).reshape(4, 128, 3 * ED).astype(BF16)
    w_out = np.ascontiguousarray(W_out).reshape(4, 128, OUT_DIM).astype(BF16)
    b_o = b_out.reshape(1, OUT_DIM).astype(np.float32)

    in_maps = []
    for c in range(NCORES):
        in_maps.append({
            "eT": np.ascontiguousarray(eT[c * NOCT:(c + 1) * NOCT]),
            "eq": np.ascontiguousarray(eq[c * NOCT:(c + 1) * NOCT]),
            "keep": np.ascontiguousarray(keep[c * NOCT:(c + 1) * NOCT]),
            "w_in": w_in,
            "w_out": w_out,
            "b_out": b_o,
        })
    return in_maps, idx, cnt


def kernel(entities, pre_mask, post_mask, W_in, W_out, b_out, trace=False):
    global LAST_RESULT
    from concourse.bass_utils import run_bass_kernel_spmd

    post_mask = np.asarray(post_mask)
    maxcnt = int((post_mask == 0).sum(axis=1).max())
    PAD = max(4, -(-maxcnt // 4) * 4)  # round up to multiple of 4

    if PAD not in _BUILT:
        _BUILT[PAD] = _build_nc(PAD)
    nc = _BUILT[PAD]

    in_maps, idx, cnt = _prep_inputs(
        np.asarray(entities), np.asarray(pre_mask), post_mask,
        np.asarray(W_in), np.asarray(W_out), np.asarray(b_out), PAD)
    res = run_bass_kernel_spmd(nc, in_maps, core_ids=list(range(NCORES)),
                               trace=trace)
    LAST_RESULT = res
    outc = np.concatenate(
        [np.asarray(r["out"]).astype(np.float32) for r in res.results], axis=0)
    out = np.zeros((BS, NQ, OUT_DIM), dtype=np.float32)
    for b in range(BS):
        n = int(cnt[b])
        if n:
            out[b, idx[b, :n], :] = outc[b, :n, :]
    return out


# revision 24
# speedup vs baseline: 1.0596x; 1.0596x over previous
"""Trainium2 Bass kernel for nn_EntityAttentionLayer (sparse attention).

Strategy (8 cores, data-parallel over bs):
  - Host side: shard bs across 8 cores (64 items each), pre-transpose
    entities to E^T[in_dim, ne] per batch, cast operands to bf16,
    convert masks to multiplicative keep-masks.
  - Query compaction: rows killed by post_mask are never computed.
    Host gathers the unmasked query entities (and their pre_mask rows)
    per batch, padded to PAD (multiple of 4, >= max unmasked count).
    All q-sized work (Q proj, logits, softmax, attn, out proj) shrinks
    64 -> PAD; the post-mask multiply disappears (kept rows are exactly
    the unmasked ones). Host scatters outputs back, zeros elsewhere.
  - On chip, per batch b (processed in pairs, Q in octets of 8):
      K^T[ed, ne]  = (Wk^T E^T)        via lhsT=Wk slices, rhs=E^T
      V[ne, ed]    = E V-proj          via lhsT=E^T slices, rhs=Wv
      Q^T[ed, q]   =                   via lhsT=Wq slices, rhs=Eq^T
      logits^T[ne, q] per head-pair    lhsT=K^T_h, rhs=Q^T (2 heads, N=2*PAD)
      wm = exp(logits * 1/sqrt(hd))    on ACT (scale folded into activation)
      wm *= keep^T                     on DVE (multiplicative mask, h-broadcast)
      sums broadcast [128, h*PAD]      PE matmul with all-ones lhsT
      attn^T unnorm [2-heads, hp*b*q]  lhsT=V slices, rhs=wm  (col-tiled pairs)
      attn = attn_unnorm * 1/sums      DVE (approx reciprocal + strided muls)
      out[b*q, out] = attn^T.T @ W_out (+bias on DVE), DMA out.
  All matmuls bf16 operands, fp32 PSUM accumulation.
"""

import numpy as np
import ml_dtypes

BS, NE, NQ, IN_DIM, ED, OUT_DIM, H, HD = 512, 256, 64, 512, 512, 512, 8, 64
NCORES = 8
BPC = BS // NCORES          # 64 batches per core
OCT = 8                     # batches per super-batch (Q^T amortization)
NOCT = BPC // OCT           # 8
PAIRS_PER_OCT = OCT // 2    # 4
NPAIRS = BPC // 2           # 32
SCALE = 1.0 / float(np.sqrt(HD))

BF16 = ml_dtypes.bfloat16

_BUILT = {}
LAST_RESULT = None


def _build_nc(PAD):
    import concourse.tile as tile
    from concourse import bacc, mybir
    from contextlib import ExitStack

    f32 = mybir.dt.float32
    bf16 = mybir.dt.bfloat16
    HQ = H * PAD            # logits/wm free size per (b2, n2)
    P2 = 2 * PAD            # head-pair logits block

    nc = bacc.Bacc("TRN2", target_bir_lowering=False)

    eT_d = nc.dram_tensor("eT", [NOCT, 128, OCT, 4, NE], bf16, kind="ExternalInput")
    eq_d = nc.dram_tensor("eq", [NOCT, 128, OCT, 4, PAD], bf16, kind="ExternalInput")
    keep_d = nc.dram_tensor("keep", [NOCT, 128, OCT, 2, PAD], bf16, kind="ExternalInput")
    w_in_d = nc.dram_tensor("w_in", [4, 128, 3 * ED], bf16, kind="ExternalInput")
    w_out_d = nc.dram_tensor("w_out", [4, 128, OUT_DIM], bf16, kind="ExternalInput")
    b_out_d = nc.dram_tensor("b_out", [1, OUT_DIM], f32, kind="ExternalInput")
    out_d = nc.dram_tensor("out", [BPC, PAD, OUT_DIM], bf16, kind="ExternalOutput")

    with ExitStack() as ctx:
        tc = ctx.enter_context(tile.TileContext(nc))
        consts = ctx.enter_context(tc.tile_pool(name="consts", bufs=1))
        p_eT = ctx.enter_context(tc.tile_pool(name="p_eT", bufs=3))
        p_eq = ctx.enter_context(tc.tile_pool(name="p_eq", bufs=3))
        p_keep = ctx.enter_context(tc.tile_pool(name="p_keep", bufs=3))
        p_kT = ctx.enter_context(tc.tile_pool(name="p_kT", bufs=3))
        p_v = ctx.enter_context(tc.tile_pool(name="p_v", bufs=3))
        p_wm = ctx.enter_context(tc.tile_pool(name="p_wm", bufs=6))
        p_recip = ctx.enter_context(tc.tile_pool(name="p_recip", bufs=2))
        p_attn = ctx.enter_context(tc.tile_pool(name="p_attn", bufs=3))
        p_out = ctx.enter_context(tc.tile_pool(name="p_out", bufs=3))
        pp = ctx.enter_context(tc.tile_pool(name="pp", bufs=2, space="PSUM"))

        # Constants. Q weights first (the first matmul needs them), in
        # 256-col chunks alternating issue engines to spread DMA queues.
        w_sb = consts.tile([128, 4, 3 * ED], bf16)
        engs = (nc.scalar, nc.gpsimd)
        for i in range(12):
            engs[i % 2].dma_start(
                out=w_sb[:, :, i * 128:(i + 1) * 128],
                in_=w_in_d[:, :, i * 128:(i + 1) * 128].rearrange("k p n -> p k n"))
        wo_sb = consts.tile([128, 4, OUT_DIM], bf16)
        nc.gpsimd.dma_start(out=wo_sb, in_=w_out_d[:, :, :].rearrange("k p n -> p k n"))
        bias_bc = consts.tile([128, OUT_DIM], f32)
        nc.gpsimd.dma_start(out=bias_bc, in_=b_out_d[:, :].to_broadcast([128, OUT_DIM]))
        ones_sb = consts.tile([128, 128], bf16)
        nc.vector.memset(ones_sb, 1.0)
        # Persistent zero-padded Q^T tiles (manual double buffer by octet
        # parity). Layout [128, m, h2, b, q]: head parity h2 selects which
        # 64-row half holds data; the other half stays zero so logits
        # matmuls can use full K=128 operands at base partition 0
        # (operands at base partition 64 fault on HW).
        qz0 = consts.tile([128, 4, 2, OCT, PAD], bf16)
        nc.vector.memset(qz0, 0.0)
        qz1 = consts.tile([128, 4, 2, OCT, PAD], bf16)
        nc.vector.memset(qz1, 0.0)
        qz_bufs = [qz0, qz1]

        def flush_out(pend):
            # out-projection of a PREVIOUS pair, emitted one pair late
            # so the PE never waits on that pair's DVE normalize.
            attn_prev, gp = pend
            ps_o = pp.tile([128, OUT_DIM], f32, tag="out", name="ps_o", bufs=1)
            for t in range(4):
                nc.tensor.matmul(
                    ps_o[0:P2, :],
                    lhsT=attn_prev[:, t * P2:(t + 1) * P2],
                    rhs=wo_sb[:, t, :],
                    start=(t == 0),
                    stop=(t == 3),
                )
            out_sb = p_out.tile([128, OUT_DIM], bf16, tag="out_sb")
            nc.vector.tensor_add(out_sb[0:P2, :], ps_o[0:P2, :], bias_bc[0:P2, :])
            for i in range(2):
                nc.sync.dma_start(
                    out=out_d[gp * 2 + i],
                    in_=out_sb[i * PAD:(i + 1) * PAD, :],
                )

        def stage_back(s):
            # attn (block-diagonal head pairs), softmax denominators and
            # normalize for a PREVIOUS pair: every input (wm, wmsum, V)
            # was produced at least one pair ago, so the PE never stalls
            # on the ACT/DVE/GPSIMD chain.
            wm, wmsum, v_sb, gp = s["wm"], s["wmsum"], s["v_sb"], s["gpair"]
            ps_a = pp.tile([128, 8 * P2], f32, tag="attn", name="ps_a", bufs=1)
            for hp in range(4):
                for b2 in range(2):
                    col = (hp * 2 + b2) * P2
                    for n2 in range(2):
                        # lhsT covers a head PAIR -> out is [128, P2] with
                        # the two wanted [64, PAD] blocks on the diagonal
                        # (off-diagonal cross-head values are ignored).
                        nc.tensor.matmul(
                            ps_a[:, col:col + P2],
                            lhsT=v_sb[:, n2, b2, hp * 128:(hp + 1) * 128],
                            rhs=wm[(b2, n2)][:, hp * P2:(hp + 1) * P2],
                            start=(n2 == 0),
                            stop=(n2 == 1),
                        )
            recip = {}
            for b2 in range(2):
                ps_s = pp.tile([128, HQ], f32, tag="logit", name="ps_s", bufs=2)
                nc.tensor.matmul(
                    ps_s, lhsT=ones_sb, rhs=wmsum[b2], start=True, stop=True)
                r_sb = p_recip.tile([128, HQ], f32, tag="recip", name="r_sb")
                nc.vector.reciprocal_approx_fast(out=r_sb, in_=ps_s)
                recip[b2] = r_sb

            # normalize -> attn_sb (bf16), layout [128(2-head rows), (hp, b2, q)]
            attn_sb = p_attn.tile([128, 4 * P2], bf16, tag="attn_sb")
            for b2 in range(2):
                for h2 in range(2):
                    rows = slice(h2 * 64, (h2 + 1) * 64)
                    o_ap = attn_sb[rows, :].rearrange(
                        "p (hp b q) -> p hp b q", hp=4, b=2)[:, :, b2, :]
                    i_ap = ps_a[rows, :].rearrange(
                        "p (hp b x) -> p hp b x", hp=4, b=2)[
                            :, :, b2, h2 * PAD:(h2 + 1) * PAD]
                    r_ap = recip[b2][rows, :].rearrange(
                        "p (hp x) -> p hp x", hp=4)[:, :, h2 * PAD:(h2 + 1) * PAD]
                    nc.vector.tensor_mul(o_ap, i_ap, r_ap)
            return (attn_sb, gp)

        back = None
        pend_q = []
        for oc in range(NOCT):
            # finer-grained DMAs on the cold first octet: per-queue DMA
            # bandwidth is ~20 GB/s, so startup latency is set by the
            # largest single transfer on the critical path.
            eqc = 4 if oc == 0 else 1
            etc = 8 if oc == 0 else 4
            eq_sb = p_eq.tile([128, OCT, 4, PAD], bf16, tag="eq")
            for i in range(eqc):
                nc.sync.dma_start(
                    out=eq_sb[:, i * (OCT // eqc):(i + 1) * (OCT // eqc), :, :],
                    in_=eq_d[oc, :, i * (OCT // eqc):(i + 1) * (OCT // eqc), :, :])
            eT_sb = p_eT.tile([128, OCT, 4, NE], bf16, tag="eT")
            for hc in range(etc):
                w = OCT // etc
                nc.sync.dma_start(
                    out=eT_sb[:, hc * w:(hc + 1) * w, :, :],
                    in_=eT_d[oc, :, hc * w:(hc + 1) * w, :, :],
                )
            keep_sb = p_keep.tile([128, OCT, 2, PAD], bf16, tag="keep")
            nc.sync.dma_start(out=keep_sb, in_=keep_d[oc, :, :, :, :])

            # ---- Q^T for the whole octet: amortize W_q weight loads ----
            qz = qz_bufs[oc % 2]
            for m in range(4):
                # logit-tag ring: keeps the K-proj's proj-ring allocation
                # independent of the qz copy drain at octet boundaries.
                ps_q = pp.tile([128, OCT * PAD], f32, tag="logit", name="ps_q", bufs=2)
                for k in range(4):
                    nc.tensor.matmul(
                        ps_q,
                        lhsT=w_sb[:, k, m * 128:(m + 1) * 128],
                        rhs=eq_sb[:, :, k, :],
                        start=(k == 0),
                        stop=(k == 3),
                    )
                nc.scalar.copy(out=qz[0:64, m, 0, :, :], in_=ps_q[0:64, :])
                nc.vector.tensor_copy(out=qz[64:128, m, 1, :, :], in_=ps_q[64:128, :])

            for pr in range(PAIRS_PER_OCT):
                lb = pr * 2          # local batch index in octet
                gpair = oc * PAIRS_PER_OCT + pr

                # ---- K^T ----
                kT_sb = p_kT.tile([128, 4, 2, NE], bf16, tag="kT")
                for m in range(4):
                    ps_k = pp.tile([128, 2 * NE], f32, tag="proj", name="ps_k", bufs=3)
                    for k in range(4):
                        nc.tensor.matmul(
                            ps_k,
                            lhsT=w_sb[:, k, ED + m * 128:ED + (m + 1) * 128],
                            rhs=eT_sb[:, lb:lb + 2, k, :],
                            start=(k == 0),
                            stop=(k == 3),
                        )
                    if m % 2 == 0:
                        nc.vector.tensor_copy(out=kT_sb[:, m, :, :], in_=ps_k)
                    else:
                        nc.scalar.copy(out=kT_sb[:, m, :, :], in_=ps_k)

                # ---- V ----
                v_sb = p_v.tile([128, 2, 2, ED], bf16, tag="v")
                for n2 in range(2):
                    for b2 in range(2):
                        ps_v = pp.tile([128, ED], f32, tag="proj", name="ps_v", bufs=3)
                        for k in range(4):
                            nc.tensor.matmul(
                                ps_v,
                                lhsT=eT_sb[:, lb + b2, k, n2 * 128:(n2 + 1) * 128],
                                rhs=w_sb[:, k, 2 * ED:3 * ED],
                                start=(k == 0),
                                stop=(k == 3),
                            )
                        if n2 == 0:
                            nc.vector.tensor_copy(out=v_sb[:, n2, b2, :], in_=ps_v)
                        else:
                            nc.scalar.copy(out=v_sb[:, n2, b2, :], in_=ps_v)

                # out-projection of pair i-2 (two pairs of DVE slack)
                if pend_q:
                    flush_out(pend_q.pop(0))

                # ---- logits^T + exp + keep-mask (this pair) ----
                # wm[(b2, n2)] : [128(ne-slice), H*PAD] bf16.  The exp /
                # keep-mul / gpsimd pre-add chain has a FULL pair of slack:
                # the attn matmuls that consume wm run one iteration later.
                wm = {}
                wmsum = {}
                for b2 in range(2):
                    for n2 in range(2):
                        ps_l = pp.tile([128, HQ], f32, tag="logit", name="ps_l", bufs=2)
                        for m in range(4):
                            nc.tensor.matmul(
                                ps_l[:, m * P2:(m + 1) * P2],
                                lhsT=kT_sb[:, m, b2, n2 * 128:(n2 + 1) * 128],
                                rhs=qz[:, m, :, lb + b2, :],
                                start=True,
                                stop=True,
                            )
                        wm_t = p_wm.tile([128, HQ], bf16, tag="wm", name="wm_t", bufs=10)
                        nc.scalar.activation(
                            out=wm_t, in_=ps_l,
                            func=mybir.ActivationFunctionType.Exp,
                            scale=SCALE,
                        )
                        keep_rep = keep_sb[:, lb + b2, n2, None, :].broadcast_to(
                            [128, H, PAD])
                        nc.vector.tensor_mul(wm_t, wm_t, keep_rep)
                        wm[(b2, n2)] = wm_t
                    ws = p_wm.tile([128, HQ], bf16, tag="wmsum", name="wmsum", bufs=4)
                    nc.gpsimd.tensor_add(ws, wm[(b2, 0)], wm[(b2, 1)])
                    wmsum[b2] = ws

                # ---- attn + denominators + normalize for pair i-1 ----
                if back is not None:
                    pend_q.append(stage_back(back))
                back = {"wm": wm, "wmsum": wmsum, "v_sb": v_sb, "gpair": gpair}

        pend_q.append(stage_back(back))
        for p in pend_q:
            flush_out(p)
    nc.compile()
    return nc


def _prep_inputs(entities, pre_mask, post_mask, W_in, W_out, b_out, PAD):
    """Host-side sharding + layout transforms (not part of timed HW work)."""
    # Query compaction: per batch, gather the post_mask==0 query rows,
    # padded (edge-repeat) to PAD. Padded columns recompute the last
    # kept query, so device output for them equals the kept row's
    # output; the scatter in kernel() only reads the first cnt rows.
    keepq = (post_mask == 0)
    cnt = keepq.sum(axis=1)
    idx = np.zeros((BS, PAD), dtype=np.int64)
    for b in range(BS):
        ib = np.nonzero(keepq[b])[0]
        if ib.size == 0:
            ib = np.zeros(1, dtype=np.int64)
        idx[b, :ib.size] = ib
        idx[b, ib.size:] = ib[-1]
    ar = np.arange(BS)[:, None]
    eqg = entities[ar, idx, :]                          # [BS, PAD, 512]
    kg = (1 - pre_mask).astype(np.float32)[ar, idx, :]  # [BS, PAD, 256]

    # [oct, 128p, b, k, ne] contiguous per partition row
    eT = entities.reshape(BS // OCT, OCT, NE, 4, 128).transpose(
        0, 4, 1, 3, 2).astype(BF16)
    eT = np.ascontiguousarray(eT)
    eq = eqg.reshape(BS // OCT, OCT, PAD, 4, 128).transpose(
        0, 4, 1, 3, 2).astype(BF16)
    eq = np.ascontiguousarray(eq)
    keep = kg.reshape(BS // OCT, OCT, PAD, 2, 128).transpose(
        0, 4, 1, 3, 2).astype(BF16)
    keep = np.ascontiguousarray(keep)
    w_in = np.ascontiguousarray(W_in).reshape(4, 128, 3 * ED).astype(BF16)
    w_out = np.ascontiguousarray(W_out).reshape(4, 128, OUT_DIM).astype(BF16)
    b_o = b_out.reshape(1, OUT_DIM).astype(np.float32)

    in_maps = []
    for c in range(NCORES):
        in_maps.append({
            "eT": np.ascontiguousarray(eT[c * NOCT:(c + 1) * NOCT]),
            "eq": np.ascontiguousarray(eq[c * NOCT:(c + 1) * NOCT]),
            "keep": np.ascontiguousarray(keep[c * NOCT:(c + 1) * NOCT]),
            "w_in": w_in,
            "w_out": w_out,
            "b_out": b_o,
        })
    return in_maps, idx, cnt


def kernel(entities, pre_mask, post_mask, W_in, W_out, b_out, trace=False):
    global LAST_RESULT
    from concourse.bass_utils import run_bass_kernel_spmd

    post_mask = np.asarray(post_mask)
    maxcnt = int((post_mask == 0).sum(axis=1).max())
    PAD = max(4, -(-maxcnt // 4) * 4)  # round up to multiple of 4

    if PAD not in _BUILT:
        _BUILT[PAD] = _build_nc(PAD)
    nc = _BUILT[PAD]

    in_maps, idx, cnt = _prep_inputs(
        np.asarray(entities), np.asarray(pre_mask), post_mask,
        np.asarray(W_in), np.asarray(W_out), np.asarray(b_out), PAD)
    res = run_bass_kernel_spmd(nc, in_maps, core_ids=list(range(NCORES)),
                               trace=trace)
    LAST_RESULT = res
    outc = np.concatenate(
        [np.asarray(r["out"]).astype(np.float32) for r in res.results], axis=0)
    out = np.zeros((BS, NQ, OUT_DIM), dtype=np.float32)
    for b in range(BS):
        n = int(cnt[b])
        if n:
            out[b, idx[b, :n], :] = outc[b, :n, :]
    return out


# revision 28
# speedup vs baseline: 1.0805x; 1.0197x over previous
"""Trainium2 Bass kernel for nn_EntityAttentionLayer (sparse attention).

Strategy (8 cores, data-parallel over bs):
  - Host side: shard bs across 8 cores (64 items each), pre-transpose
    entities to E^T[in_dim, ne] per batch, cast operands to bf16,
    convert masks to multiplicative keep-masks.
  - Query compaction: rows killed by post_mask are never computed.
    Host gathers the unmasked query entities (and their pre_mask rows)
    per batch, padded to PAD (multiple of 4, >= max unmasked count).
    All q-sized work (Q proj, logits, softmax, attn, out proj) shrinks
    64 -> PAD; the post-mask multiply disappears (kept rows are exactly
    the unmasked ones). Host scatters outputs back, zeros elsewhere.
  - On chip, per batch b (processed in pairs, Q in octets of 8):
      K^T[ed, ne]  = (Wk^T E^T)        via lhsT=Wk slices, rhs=E^T
      V[ne, ed]    = E V-proj          via lhsT=E^T slices, rhs=Wv
      Q^T[ed, q]   =                   via lhsT=Wq slices, rhs=Eq^T
      logits^T[ne, q] per head-pair    lhsT=K^T_h, rhs=Q^T (2 heads, N=2*PAD)
      wm = exp(logits * 1/sqrt(hd))    on ACT (scale folded into activation)
      wm *= keep^T                     on DVE (multiplicative mask, h-broadcast)
      sums broadcast [128, h*PAD]      PE matmul with all-ones lhsT
      attn^T unnorm [2-heads, hp*b*q]  lhsT=V slices, rhs=wm  (col-tiled pairs)
      attn = attn_unnorm * 1/sums      DVE (approx reciprocal + strided muls)
      out[b*q, out] = attn^T.T @ W_out (+bias on DVE), DMA out.
  All matmuls bf16 operands, fp32 PSUM accumulation.
"""

import numpy as np
import ml_dtypes

BS, NE, NQ, IN_DIM, ED, OUT_DIM, H, HD = 512, 256, 64, 512, 512, 512, 8, 64
NCORES = 8
BPC = BS // NCORES          # 64 batches per core
OCT = 8                     # batches per super-batch (Q^T amortization)
NOCT = BPC // OCT           # 8
PAIRS_PER_OCT = OCT // 2    # 4
NPAIRS = BPC // 2           # 32
SCALE = 1.0 / float(np.sqrt(HD))

BF16 = ml_dtypes.bfloat16

_BUILT = {}
LAST_RESULT = None


def _build_nc(PAD):
    import concourse.tile as tile
    from concourse import bacc, mybir
    from contextlib import ExitStack

    f32 = mybir.dt.float32
    bf16 = mybir.dt.bfloat16
    HQ = H * PAD            # logits/wm free size per (b2, n2)
    P2 = 2 * PAD            # head-pair logits block

    nc = bacc.Bacc("TRN2", target_bir_lowering=False)

    eT_d = nc.dram_tensor("eT", [NOCT, 128, OCT, 4, NE], bf16, kind="ExternalInput")
    eq_d = nc.dram_tensor("eq", [NOCT, 128, OCT, 4, PAD], bf16, kind="ExternalInput")
    keep_d = nc.dram_tensor("keep", [NOCT, 128, OCT, 2, PAD], bf16, kind="ExternalInput")
    w_in_d = nc.dram_tensor("w_in", [4, 128, 3 * ED], bf16, kind="ExternalInput")
    w_out_d = nc.dram_tensor("w_out", [4, 128, OUT_DIM], bf16, kind="ExternalInput")
    b_out_d = nc.dram_tensor("b_out", [1, OUT_DIM], f32, kind="ExternalInput")
    out_d = nc.dram_tensor("out", [BPC, PAD, OUT_DIM], bf16, kind="ExternalOutput")

    with ExitStack() as ctx:
        tc = ctx.enter_context(tile.TileContext(nc))
        consts = ctx.enter_context(tc.tile_pool(name="consts", bufs=1))
        p_eT = ctx.enter_context(tc.tile_pool(name="p_eT", bufs=3))
        p_eq = ctx.enter_context(tc.tile_pool(name="p_eq", bufs=3))
        p_keep = ctx.enter_context(tc.tile_pool(name="p_keep", bufs=3))
        p_kT = ctx.enter_context(tc.tile_pool(name="p_kT", bufs=3))
        p_v = ctx.enter_context(tc.tile_pool(name="p_v", bufs=3))
        p_wm = ctx.enter_context(tc.tile_pool(name="p_wm", bufs=6))
        p_recip = ctx.enter_context(tc.tile_pool(name="p_recip", bufs=2))
        p_attn = ctx.enter_context(tc.tile_pool(name="p_attn", bufs=3))
        p_out = ctx.enter_context(tc.tile_pool(name="p_out", bufs=3))
        pp = ctx.enter_context(tc.tile_pool(name="pp", bufs=2, space="PSUM"))

        # Constants. Each dma_start costs ~0.7us of sequencer issue time
        # and its descriptors spray across all 16 DMA queues, so use FEW
        # transfers, issued in need-order (Q weights first).
        w_sb = consts.tile([128, 4, 3 * ED], bf16)
        engs = (nc.scalar, nc.gpsimd)
        for i in range(3):
            engs[i % 2].dma_start(
                out=w_sb[:, :, i * ED:(i + 1) * ED],
                in_=w_in_d[:, :, i * ED:(i + 1) * ED].rearrange("k p n -> p k n"))
        wo_sb = consts.tile([128, 4, OUT_DIM], bf16)
        nc.gpsimd.dma_start(out=wo_sb, in_=w_out_d[:, :, :].rearrange("k p n -> p k n"))
        bias_bc = consts.tile([128, OUT_DIM], f32)
        nc.gpsimd.dma_start(out=bias_bc, in_=b_out_d[:, :].to_broadcast([128, OUT_DIM]))
        ones_sb = consts.tile([128, 128], bf16)
        nc.vector.memset(ones_sb, 1.0)
        # Persistent zero-padded Q^T tiles (manual double buffer by octet
        # parity). Layout [128, m, h2, b, q]: head parity h2 selects which
        # 64-row half holds data; the other half stays zero so logits
        # matmuls can use full K=128 operands at base partition 0
        # (operands at base partition 64 fault on HW).
        qz0 = consts.tile([128, 4, 2, OCT, PAD], bf16)
        nc.vector.memset(qz0, 0.0)
        qz1 = consts.tile([128, 4, 2, OCT, PAD], bf16)
        nc.vector.memset(qz1, 0.0)
        qz_bufs = [qz0, qz1]

        def flush_out(pend):
            # out-projection of a PREVIOUS pair, emitted one pair late
            # so the PE never waits on that pair's DVE normalize.
            attn_prev, gp = pend
            ps_o = pp.tile([128, OUT_DIM], f32, tag="out", name="ps_o", bufs=1)
            for t in range(4):
                nc.tensor.matmul(
                    ps_o[0:P2, :],
                    lhsT=attn_prev[:, t * P2:(t + 1) * P2],
                    rhs=wo_sb[:, t, :],
                    start=(t == 0),
                    stop=(t == 3),
                )
            out_sb = p_out.tile([128, OUT_DIM], bf16, tag="out_sb")
            nc.vector.tensor_add(out_sb[0:P2, :], ps_o[0:P2, :], bias_bc[0:P2, :])
            nc.sync.dma_start(
                out=out_d[gp * 2:gp * 2 + 2].rearrange("b q d -> (b q) d"),
                in_=out_sb[0:P2, :],
            )

        def stage_back(s):
            # attn (block-diagonal head pairs), softmax denominators and
            # normalize for a PREVIOUS pair: every input (wm, wmsum, V)
            # was produced at least one pair ago, so the PE never stalls
            # on the ACT/DVE/GPSIMD chain.
            wm, wmsum, v_sb, gp = s["wm"], s["wmsum"], s["v_sb"], s["gpair"]
            ps_a = pp.tile([128, 8 * P2], f32, tag="attn", name="ps_a", bufs=1)
            for hp in range(4):
                for b2 in range(2):
                    col = (hp * 2 + b2) * P2
                    for n2 in range(2):
                        # lhsT covers a head PAIR -> out is [128, P2] with
                        # the two wanted [64, PAD] blocks on the diagonal
                        # (off-diagonal cross-head values are ignored).
                        nc.tensor.matmul(
                            ps_a[:, col:col + P2],
                            lhsT=v_sb[:, n2, b2, hp * 128:(hp + 1) * 128],
                            rhs=wm[(b2, n2)][:, hp * P2:(hp + 1) * P2],
                            start=(n2 == 0),
                            stop=(n2 == 1),
                        )
            recip = {}
            for b2 in range(2):
                # proj ring: keeps the octet-boundary Q-proj (logit ring)
                # from waiting on this tile's DVE reciprocal drain.
                ps_s = pp.tile([128, HQ], f32, tag="proj", name="ps_s", bufs=3)
                nc.tensor.matmul(
                    ps_s, lhsT=ones_sb, rhs=wmsum[b2], start=True, stop=True)
                r_sb = p_recip.tile([128, HQ], f32, tag="recip", name="r_sb")
                nc.vector.reciprocal_approx_fast(out=r_sb, in_=ps_s)
                recip[b2] = r_sb

            # normalize -> attn_sb (bf16), layout [128(2-head rows), (hp, b2, q)]
            attn_sb = p_attn.tile([128, 4 * P2], bf16, tag="attn_sb")
            for b2 in range(2):
                for h2 in range(2):
                    rows = slice(h2 * 64, (h2 + 1) * 64)
                    o_ap = attn_sb[rows, :].rearrange(
                        "p (hp b q) -> p hp b q", hp=4, b=2)[:, :, b2, :]
                    i_ap = ps_a[rows, :].rearrange(
                        "p (hp b x) -> p hp b x", hp=4, b=2)[
                            :, :, b2, h2 * PAD:(h2 + 1) * PAD]
                    r_ap = recip[b2][rows, :].rearrange(
                        "p (hp x) -> p hp x", hp=4)[:, :, h2 * PAD:(h2 + 1) * PAD]
                    nc.vector.tensor_mul(o_ap, i_ap, r_ap)
            return (attn_sb, gp)

        back = None
        pend_q = []
        for oc in range(NOCT):
            # Need-ordered: eq (first matmul), first eT pair (K-proj of
            # pair 0), rest of eT, keep (keep-mul comes last).
            eq_sb = p_eq.tile([128, OCT, 4, PAD], bf16, tag="eq")
            nc.sync.dma_start(out=eq_sb, in_=eq_d[oc, :, :, :, :])
            eT_sb = p_eT.tile([128, OCT, 4, NE], bf16, tag="eT")
            nc.sync.dma_start(out=eT_sb[:, 0:2, :, :], in_=eT_d[oc, :, 0:2, :, :])
            nc.sync.dma_start(out=eT_sb[:, 2:8, :, :], in_=eT_d[oc, :, 2:8, :, :])
            keep_sb = p_keep.tile([128, OCT, 2, PAD], bf16, tag="keep")
            nc.sync.dma_start(out=keep_sb, in_=keep_d[oc, :, :, :, :])

            # ---- Q^T for the whole octet: amortize W_q weight loads ----
            qz = qz_bufs[oc % 2]
            for m in range(4):
                # logit-tag ring: keeps the K-proj's proj-ring allocation
                # independent of the qz copy drain at octet boundaries.
                ps_q = pp.tile([128, OCT * PAD], f32, tag="logit", name="ps_q", bufs=2)
                for k in range(4):
                    nc.tensor.matmul(
                        ps_q,
                        lhsT=w_sb[:, k, m * 128:(m + 1) * 128],
                        rhs=eq_sb[:, :, k, :],
                        start=(k == 0),
                        stop=(k == 3),
                    )
                nc.scalar.copy(out=qz[0:64, m, 0, :, :], in_=ps_q[0:64, :])
                nc.vector.tensor_copy(out=qz[64:128, m, 1, :, :], in_=ps_q[64:128, :])

            for pr in range(PAIRS_PER_OCT):
                lb = pr * 2          # local batch index in octet
                gpair = oc * PAIRS_PER_OCT + pr

                # ---- K^T ----
                kT_sb = p_kT.tile([128, 4, 2, NE], bf16, tag="kT")
                for m in range(4):
                    ps_k = pp.tile([128, 2 * NE], f32, tag="proj", name="ps_k", bufs=3)
                    for k in range(4):
                        nc.tensor.matmul(
                            ps_k,
                            lhsT=w_sb[:, k, ED + m * 128:ED + (m + 1) * 128],
                            rhs=eT_sb[:, lb:lb + 2, k, :],
                            start=(k == 0),
                            stop=(k == 3),
                        )
                    if m % 2 == 0:
                        nc.vector.tensor_copy(out=kT_sb[:, m, :, :], in_=ps_k)
                    else:
                        nc.scalar.copy(out=kT_sb[:, m, :, :], in_=ps_k)

                # ---- V ----
                v_sb = p_v.tile([128, 2, 2, ED], bf16, tag="v")
                for n2 in range(2):
                    for b2 in range(2):
                        ps_v = pp.tile([128, ED], f32, tag="proj", name="ps_v", bufs=3)
                        for k in range(4):
                            nc.tensor.matmul(
                                ps_v,
                                lhsT=eT_sb[:, lb + b2, k, n2 * 128:(n2 + 1) * 128],
                                rhs=w_sb[:, k, 2 * ED:3 * ED],
                                start=(k == 0),
                                stop=(k == 3),
                            )
                        if n2 == 0:
                            nc.vector.tensor_copy(out=v_sb[:, n2, b2, :], in_=ps_v)
                        else:
                            nc.scalar.copy(out=v_sb[:, n2, b2, :], in_=ps_v)

                # out-projection of pair i-2 (two pairs of DVE slack)
                if pend_q:
                    flush_out(pend_q.pop(0))

                # ---- logits^T + exp + keep-mask (this pair) ----
                # wm[(b2, n2)] : [128(ne-slice), H*PAD] bf16.  The exp /
                # keep-mul / gpsimd pre-add chain has a FULL pair of slack:
                # the attn matmuls that consume wm run one iteration later.
                wm = {}
                wmsum = {}
                for b2 in range(2):
                    for n2 in range(2):
                        ps_l = pp.tile([128, HQ], f32, tag="logit", name="ps_l", bufs=2)
                        for m in range(4):
                            nc.tensor.matmul(
                                ps_l[:, m * P2:(m + 1) * P2],
                                lhsT=kT_sb[:, m, b2, n2 * 128:(n2 + 1) * 128],
                                rhs=qz[:, m, :, lb + b2, :],
                                start=True,
                                stop=True,
                            )
                        wm_t = p_wm.tile([128, HQ], bf16, tag="wm", name="wm_t", bufs=10)
                        nc.scalar.activation(
                            out=wm_t, in_=ps_l,
                            func=mybir.ActivationFunctionType.Exp,
                            scale=SCALE,
                        )
                        keep_rep = keep_sb[:, lb + b2, n2, None, :].broadcast_to(
                            [128, H, PAD])
                        nc.vector.tensor_mul(wm_t, wm_t, keep_rep)
                        wm[(b2, n2)] = wm_t
                    ws = p_wm.tile([128, HQ], bf16, tag="wmsum", name="wmsum", bufs=4)
                    nc.gpsimd.tensor_add(ws, wm[(b2, 0)], wm[(b2, 1)])
                    wmsum[b2] = ws

                # ---- attn + denominators + normalize for pair i-1 ----
                if back is not None:
                    pend_q.append(stage_back(back))
                back = {"wm": wm, "wmsum": wmsum, "v_sb": v_sb, "gpair": gpair}

        pend_q.append(stage_back(back))
        for p in pend_q:
            flush_out(p)
    nc.compile()
    return nc


def _prep_inputs(entities, pre_mask, post_mask, W_in, W_out, b_out, PAD):
    """Host-side sharding + layout transforms (not part of timed HW work)."""
    # Query compaction: per batch, gather the post_mask==0 query rows,
    # padded (edge-repeat) to PAD. Padded columns recompute the last
    # kept query, so device output for them equals the kept row's
    # output; the scatter in kernel() only reads the first cnt rows.
    keepq = (post_mask == 0)
    cnt = keepq.sum(axis=1)
    idx = np.zeros((BS, PAD), dtype=np.int64)
    for b in range(BS):
        ib = np.nonzero(keepq[b])[0]
        if ib.size == 0:
            ib = np.zeros(1, dtype=np.int64)
        idx[b, :ib.size] = ib
        idx[b, ib.size:] = ib[-1]
    ar = np.arange(BS)[:, None]
    eqg = entities[ar, idx, :]                          # [BS, PAD, 512]
    kg = (1 - pre_mask).astype(np.float32)[ar, idx, :]  # [BS, PAD, 256]

    # [oct, 128p, b, k, ne] contiguous per partition row
    eT = entities.reshape(BS // OCT, OCT, NE, 4, 128).transpose(
        0, 4, 1, 3, 2).astype(BF16)
    eT = np.ascontiguousarray(eT)
    eq = eqg.reshape(BS // OCT, OCT, PAD, 4, 128).transpose(
        0, 4, 1, 3, 2).astype(BF16)
    eq = np.ascontiguousarray(eq)
    keep = kg.reshape(BS // OCT, OCT, PAD, 2, 128).transpose(
        0, 4, 1, 3, 2).astype(BF16)
    keep = np.ascontiguousarray(keep)
    w_in = np.ascontiguousarray(W_in).reshape(4, 128, 3 * ED).astype(BF16)
    w_out = np.ascontiguousarray(W_out).reshape(4, 128, OUT_DIM).astype(BF16)
    b_o = b_out.reshape(1, OUT_DIM).astype(np.float32)

    in_maps = []
    for c in range(NCORES):
        in_maps.append({
            "eT": np.ascontiguousarray(eT[c * NOCT:(c + 1) * NOCT]),
            "eq": np.ascontiguousarray(eq[c * NOCT:(c + 1) * NOCT]),
            "keep": np.ascontiguousarray(keep[c * NOCT:(c + 1) * NOCT]),
            "w_in": w_in,
            "w_out": w_out,
            "b_out": b_o,
        })
    return in_maps, idx, cnt


def kernel(entities, pre_mask, post_mask, W_in, W_out, b_out, trace=False):
    global LAST_RESULT
    from concourse.bass_utils import run_bass_kernel_spmd

    post_mask = np.asarray(post_mask)
    maxcnt = int((post_mask == 0).sum(axis=1).max())
    PAD = max(4, -(-maxcnt // 4) * 4)  # round up to multiple of 4

    if PAD not in _BUILT:
        _BUILT[PAD] = _build_nc(PAD)
    nc = _BUILT[PAD]

    in_maps, idx, cnt = _prep_inputs(
        np.asarray(entities), np.asarray(pre_mask), post_mask,
        np.asarray(W_in), np.asarray(W_out), np.asarray(b_out), PAD)
    res = run_bass_kernel_spmd(nc, in_maps, core_ids=list(range(NCORES)),
                               trace=trace)
    LAST_RESULT = res
    outc = np.concatenate(
        [np.asarray(r["out"]).astype(np.float32) for r in res.results], axis=0)
    out = np.zeros((BS, NQ, OUT_DIM), dtype=np.float32)
    for b in range(BS):
        n = int(cnt[b])
        if n:
            out[b, idx[b, :n], :] = outc[b, :n, :]
    return out
